# revision 1
# baseline (speedup 1.0000x reference)
"""Trainium2 Bass kernel for nn_CompositeEmbeddingA (octree composite embedding).

Per sample (1 sample per NeuronCore, batch=8 over 8 cores):
  layers 0-2 (depths 1-3): x = val_emb[v] + pos0[p0] + pos1[p1] + pos2[p2] + dep_emb[d]
  layers 3-4: same sum w/o dep, then Conv1d(E,E,kernel=stride=k), k=4 (l3) / 8 (l4)

Algorithm: every layer is expressed as  out = MultiHot @ Table  on the PE:
  - conv folded into the tables host-side: per tap j, T_j = concat(tables) @ w[:,:,j].T,
    so out[t] = sum_j multihot(token 8t+j) @ T_j  == one K=(196k) matmul per layer.
  - MultiHot^T (contraction dim on partitions) is built on-chip:
      PE "broadcast matmul": bcast[r_row, tok] = selector^T @ idx_rows  (replicates the
      right index value into every table row), then DVE is_equal against a per-partition
      constant column -> exact 0/1 one-hot, fp32.
  - conv bias = one extra table row whose selector column is all-zero (bcast value 0)
    with compare const 0 -> fires for every token.
  - main matmuls run in float32r (full fp32 data, 1 cycle/row at N>=256).
"""

import sys

for _p in ("/opt/trn_rl_repo",):
    if _p not in sys.path:
        sys.path.insert(0, _p)

import numpy as np
import ml_dtypes

RES = 32
SPATIAL = 3
NUM_VOCAB = 3
E = 256
BATCH = 8
LAYER_SIZES = (8, 64, 512, 4096, 32768)
CONV_SIZE = {3: 4, 4: 8}
S_TOTAL = sum(LAYER_SIZES)  # 37448
OUT_TOKENS = 8 + 64 + 512 + 1024 + 4096  # 5704
NIDX = 33  # 32 idx rows + one all-ones row (carries the -c compare constants)
ONES_ROW = 32
STRIPE = 512

# segment widths inside one tap: value(4), pos0(64), pos1(64), pos2(64) [, dep(6)]
SEG_W = (NUM_VOCAB + 1, 2 * RES, 2 * RES, 2 * RES)
DEP_W = 6

_BF16 = ml_dtypes.bfloat16


def _layer_slices():
    out = []
    start = 0
    for n in LAYER_SIZES:
        out.append((start, start + n))
        start += n
    return out


LAYER_SL = _layer_slices()


def _build_consts(params):
    """Fold conv weights into tables; pack rows into 128-row chunks.

    Returns (tbl [NC,128,256] f32, sel [NC,32,128] bf16, cval [NC,128,1] f32,
             layers: list of (name, T_tokens, out_offset, chunk_index_list))
    """
    rows_tbl = []   # per logical row: the 256-vector
    rows_ridx = []  # which of the 32 idx rows feeds this row (-1 = none: bcast val 0)
    rows_c = []     # compare constant
    layer_marks = []  # (row_start, row_end) per virtual layer

    def seg_tables(l):
        t = [np.asarray(params[f"val_emb_{l}"], np.float32)]
        pe = np.asarray(params[f"pos_emb_{l}"], np.float32)
        t += [pe[0], pe[1], pe[2]]
        return t

    # virtual layer "B": real layers 0..2 merged. idx rows: l*5 + (v,p0,p1,p2,d)
    r0 = len(rows_tbl)
    for l in range(3):
        tabs = seg_tables(l) + [np.asarray(params[f"dep_emb_{l}"], np.float32)]
        for seg, tab in enumerate(tabs):
            for c in range(tab.shape[0]):
                rows_tbl.append(tab[c])
                rows_ridx.append(l * 5 + seg)
                rows_c.append(float(c))
    layer_marks.append((r0, len(rows_tbl)))

    # conv layers: idx rows j*4+seg; one bias row (all-zero selector col, c=0)
    for l in (3, 4):
        r0 = len(rows_tbl)
        k = CONV_SIZE[l]
        w = np.asarray(params[f"conv_w_{l}"], np.float32)  # [O, E, k]
        b = np.asarray(params[f"conv_b_{l}"], np.float32)  # [O]
        tabs = seg_tables(l)
        for j in range(k):
            wj = w[:, :, j]  # [O, E]
            for seg, tab in enumerate(tabs):
                folded = tab @ wj.T  # [rows, O]
                for c in range(tab.shape[0]):
                    rows_tbl.append(folded[c])
                    rows_ridx.append(j * 4 + seg)
                    rows_c.append(float(c))
        rows_tbl.append(b)
        rows_ridx.append(-1)
        rows_c.append(0.0)
        layer_marks.append((r0, len(rows_tbl)))

    # chunkify each virtual layer into 128-row chunks
    tbl_chunks, sel_chunks, cval_chunks = [], [], []
    layers = []
    out_offs = [0, 584, 1608]
    names = ["B", "L3", "L4"]
    t_counts = [584, 1024, 4096]
    for vl, (r0, r1) in enumerate(layer_marks):
        n = r1 - r0
        nch = -(-n // 128)
        cids = []
        for ci in range(nch):
            a = r0 + ci * 128
            bnd = min(r0 + (ci + 1) * 128, r1)
            rows = bnd - a
            tbl = np.zeros((128, E), np.float32)
            sel = np.zeros((NIDX, 128), np.float32)
            sel[ONES_ROW, :] = 1.0  # pad rows: bcast value = +1 -> eq(.,0)=0
            for m in range(rows):
                tbl[m] = rows_tbl[a + m]
                if rows_ridx[a + m] >= 0:
                    sel[rows_ridx[a + m], m] = 1.0
                # ones-row coefficient: broadcast out = idx - c
                sel[ONES_ROW, m] = -rows_c[a + m]
            cids.append(len(tbl_chunks))
            tbl_chunks.append(tbl)
            sel_chunks.append(sel.astype(_BF16))
        layers.append((names[vl], t_counts[vl], out_offs[vl], cids))

    # merged layouts: one DMA per constant tensor
    tbl = np.concatenate(tbl_chunks, axis=1)  # [128, NC*256] f32
    sel = np.concatenate(sel_chunks, axis=1)  # [33, NC*128] bf16
    return tbl, sel, layers


def _build_ridx(value, depth, position, b):
    """Per-core index-row tensors, one per virtual layer: [32, T] bf16."""
    out = {}
    # B: merged layers 0-2; out tokens 0..583 = input tokens 0..583
    rb = np.full((NIDX, 584), -1.0, np.float32)
    rb[ONES_ROW] = 1.0
    col = 0
    for l in range(3):
        lo, hi = LAYER_SL[l]
        n = hi - lo
        rb[l * 5 + 0, col : col + n] = value[b, lo:hi]
        for s in range(3):
            rb[l * 5 + 1 + s, col : col + n] = position[b, lo:hi, s]
        rb[l * 5 + 4, col : col + n] = depth[b, lo:hi]
        col += n
    out["B"] = rb.astype(_BF16)
    for name, l in (("L3", 3), ("L4", 4)):
        k = CONV_SIZE[l]
        lo, hi = LAYER_SL[l]
        T = (hi - lo) // k
        r = np.zeros((NIDX, T), np.float32)
        r[ONES_ROW] = 1.0
        for j in range(k):
            r[j * 4 + 0] = value[b, lo:hi][j::k]
            for s in range(3):
                r[j * 4 + 1 + s] = position[b, lo:hi, s][j::k]
        out[name] = r.astype(_BF16)
    return out


_CACHE = {}

# schedule tuning knobs (sweepable via analyze_sweep.py)
PAIR = 1  # chunks fused per eq op
BPS_BUFS = 5
OPS_BUFS = 3
MH_BUFS = 3
ACT_MOD = 4  # pair p goes to ACT when p % ACT_MOD == ACT_MOD - 1
DEPTH = 2
STAGE = "full"  # "full" | "mh_only" | "main_only" (HW bisection)
EQ_BF16 = False  # bf16 PSUM matmul output is TRN3-only
TT_PAIR = 1  # main t-tiles packed per PSUM bank (2 regressed on HW: 311us)


def _get_nc(layers, nchunks, reps=1):
    key = ("v1", PAIR, BPS_BUFS, OPS_BUFS, MH_BUFS, ACT_MOD, DEPTH, reps, STAGE,
           EQ_BF16, TT_PAIR, tuple((n, t, o, tuple(c)) for n, t, o, c in layers))
    if key in _CACHE:
        return _CACHE[key]

    import concourse.bass as bass
    import concourse.tile as tile
    from concourse import bacc, mybir
    from contextlib import ExitStack

    f32 = mybir.dt.float32
    f32r = mybir.dt.float32r
    bf16 = mybir.dt.bfloat16

    nc = bacc.Bacc(trn_type="TRN2", target_bir_lowering=False, debug=False)
    tbl_d = nc.dram_tensor("tbl", [128, nchunks * E], f32r, kind="ExternalInput").ap()
    sel_d = nc.dram_tensor(
        "sel", [NIDX, nchunks * 128], bf16, kind="ExternalInput"
    ).ap()
    ridx_d = {
        name: nc.dram_tensor(f"ridx_{name}", [NIDX, T], bf16, kind="ExternalInput").ap()
        for name, T, _, _ in layers
    }
    out_d = nc.dram_tensor("out", [OUT_TOKENS, E], f32, kind="ExternalOutput").ap()

    with tile.TileContext(nc) as tc, ExitStack() as ctx:
        cpool = ctx.enter_context(tc.tile_pool(name="const", bufs=1))
        rpool = ctx.enter_context(tc.tile_pool(name="ridx", bufs=DEPTH + 1))
        mpool = ctx.enter_context(tc.tile_pool(name="mh", bufs=MH_BUFS))
        tpool = ctx.enter_context(tc.tile_pool(name="sq", bufs=3))
        bps = ctx.enter_context(
            tc.tile_pool(name="bps", bufs=BPS_BUFS, space=bass.MemorySpace.PSUM)
        )
        ops = ctx.enter_context(
            tc.tile_pool(name="ops", bufs=OPS_BUFS, space=bass.MemorySpace.PSUM)
        )
        opool = ctx.enter_context(tc.tile_pool(name="osb", bufs=3))

        # small consts first so the first broadcast matmuls start immediately;
        # the big table load is split per-layer in use order behind them
        sel_t = cpool.tile([NIDX, nchunks * 128], bf16, tag="sel")
        nc.sync.dma_start(sel_t[:], sel_d[:])
        tbl_t = cpool.tile([128, nchunks * E], f32r, tag="tbl")
        for _, _, _, cids in layers:
            lo, hi = cids[0] * E, (cids[-1] + 1) * E
            nc.sync.dma_start(tbl_t[:, lo:hi], tbl_d[:, lo:hi])

        A = mybir.ActivationFunctionType
        stripes = []
        for name, T, out_off, cids in layers:
            for s0 in range(0, T, STRIPE):
                stripes.append((name, out_off, cids, s0, min(STRIPE, T - s0)))
        # spread the small eq-heavy stripes (B/L3) between PE-heavy L4 ones
        big = [s for s in stripes if s[0] == "L4"]
        small = [s for s in stripes if s[0] != "L4"]
        small.sort(key=lambda s: -s[4])  # tiny tail stripe goes last
        stripes = []
        for i, b in enumerate(big):
            stripes.append(b)
            if i * len(small) // len(big) < (i + 1) * len(small) // len(big):
                stripes.append(small[i * len(small) // len(big)])

        def load_ridx(si):
            name, _, cids, s0, W = stripes[si]
            rt = rpool.tile([NIDX, W], bf16, tag="r")
            nc.sync.dma_start(rt[:], ridx_d[name][:, s0 : s0 + W])
            return rt

        def build_mh_pair(si, rt, p, ks):
            """broadcast matmuls + eq for a pair (or single) of chunks.

            The broadcast output is already idx - c (ones-row trick), so the
            one-hot is a compare against immediate 0 and one DVE/ACT op can
            span both chunks of the pair.
            """
            _, _, cids, _, W = stripes[si]
            n = len(ks)
            bp = bps.tile([128, n * W], bf16 if EQ_BF16 else f32, tag="b")
            for i, k in enumerate(ks):
                ci = cids[k]
                nc.tensor.matmul(
                    bp[:, i * W : (i + 1) * W],
                    sel_t[:, ci * 128 : (ci + 1) * 128],
                    rt[:],
                    start=True,
                    stop=True,
                )
            mh = mpool.tile([128, n * W], f32r, tag=f"mh{p}")
            if p % ACT_MOD == ACT_MOD - 1:
                # ACT path: relu(1 - x^2) — exact 0/1 for integer x
                tmp = tpool.tile([128, n * W], f32, tag="sq")
                nc.scalar.activation(tmp[:], bp[:], A.Square)
                nc.scalar.activation(mh[:], tmp[:], A.Relu, bias=1.0, scale=-1.0)
            else:
                nc.vector.tensor_scalar(
                    mh[:], bp[:], 0.0, None, op0=mybir.AluOpType.is_equal
                )
            return [mh[:, i * W : (i + 1) * W] for i in range(n)]

        def main_ttile(si, mhs, ti, ob):
            """two t-tiles packed into one PSUM bank; one evict per pair."""
            _, _, cids, _, W = stripes[si]
            nt = min(TT_PAIR, -(-W // 128) - TT_PAIR * ti)
            op = ops.tile([128, nt * E], f32, tag="o")
            Ms = []
            for h in range(nt):
                t0 = (TT_PAIR * ti + h) * 128
                M = min(128, W - t0)
                Ms.append(M)
                for k, ci in enumerate(cids):
                    nc.tensor.matmul(
                        op[:M, h * E : h * E + E],
                        mhs[k][:, t0 : t0 + M],
                        tbl_t[:, ci * E : (ci + 1) * E],
                        start=(k == 0),
                        stop=(k == len(cids) - 1),
                    )
            col = TT_PAIR * ti * E
            if nt == 2 and Ms[0] == 128 and Ms[1] == 128:
                nc.scalar.activation(ob[:, col : col + 2 * E], op[:], A.Copy)
            else:
                for h in range(nt):
                    nc.scalar.activation(
                        ob[: Ms[h], col + h * E : col + (h + 1) * E],
                        op[: Ms[h], h * E : h * E + E],
                        A.Copy,
                    )

        def store_out(si, ob):
            _, out_off, _, s0, W = stripes[si]
            row = out_off + s0
            if W % 128 == 0:
                dst = out_d[row : row + W, :].rearrange("(a p) e -> p a e", p=128)
                src = ob[:].rearrange("p (a e) -> p a e", e=E)
                nc.sync.dma_start(dst, src)
            else:
                nc.sync.dma_start(out_d[row : row + W, :], ob[:W, :E])

        # two-stripe software pipeline with interleaved emission: pair-builds
        # of stripe s+2's one-hots alternate with stripe s's main t-tiles.
        def stripe_pairs(si):
            nk = len(stripes[si][2])
            return [tuple(range(a, min(a + PAIR, nk))) for a in range(0, nk, PAIR)]

        def emit_pairs(si, rt, prs):
            mhs = []
            for p, ks in prs:
                mhs += build_mh_pair(si, rt, p, ks)
            return mhs

        def emit_body_mh_only():
            for si in range(len(stripes)):
                rt = load_ridx(si)
                emit_pairs(si, rt, list(enumerate(stripe_pairs(si))))

        static_mh = {}
        if STAGE == "main_only":
            tmp0 = cpool.tile([128, STRIPE], f32, tag="smhtmp")
            nc.gpsimd.memset(tmp0[:], 0.5)
            for p in range(13):
                t = cpool.tile([128, STRIPE], f32r, tag=f"smh{p}")
                nc.vector.tensor_scalar(
                    t[:], tmp0[:], 0.0, None, op0=mybir.AluOpType.is_equal
                )
                static_mh[p] = t

        def emit_body_main_only():
            for si in range(len(stripes)):
                _, _, cids, _, W = stripes[si]
                ntt = -(-W // 128)
                ob = opool.tile([128, ntt * E], f32, tag="ob")
                mhs = [static_mh[k][:, :W] for k in range(len(cids))]
                for ti in range(-(-ntt // TT_PAIR)):
                    main_ttile(si, mhs, ti, ob)
                store_out(si, ob)

        def emit_body():
            nst = len(stripes)
            mh_of = {}
            for si in range(min(DEPTH, nst)):
                rt = load_ridx(si)
                mh_of[si] = emit_pairs(si, rt, list(enumerate(stripe_pairs(si))))
            for si in range(nst):
                W = stripes[si][4]
                ntt = -(-W // 128)
                ngr = -(-ntt // TT_PAIR)
                ob = opool.tile([128, ntt * E], f32, tag="ob")
                sj = si + DEPTH
                if sj < nst:
                    rt = load_ridx(sj)
                    prs = list(enumerate(stripe_pairs(sj)))
                    npr = len(prs)
                    # split stripe sj's pair-builds into groups interleaved
                    # with stripe si's main t-tile pairs
                    bounds = [round(g * npr / ngr) for g in range(ngr + 1)]
                    mh_of[sj] = []
                    for ti in range(ngr):
                        main_ttile(si, mh_of[si], ti, ob)
                        mh_of[sj] += emit_pairs(
                            sj, rt, prs[bounds[ti] : bounds[ti + 1]]
                        )
                else:
                    for ti in range(ngr):
                        main_ttile(si, mh_of[si], ti, ob)
                store_out(si, ob)
                del mh_of[si]

        body_fn = {
            "full": emit_body,
            "mh_only": emit_body_mh_only,
            "main_only": emit_body_main_only,
        }[STAGE]
        if reps == 1:
            body_fn()
        else:
            # timing mode: repeat the body on-device to measure per-iter HW
            # time as a wall-clock slope (no NTFF profiling available)
            hints = (
                mybir.EngineType.PE,
                mybir.EngineType.DVE,
                mybir.EngineType.Activation,
                mybir.EngineType.SP,
            )
            with tc.For_i(0, reps, 1, hint_engines=hints):
                body_fn()

    nc.compile()
    _CACHE[key] = nc
    return nc


def kernel(**inputs):
    from concourse.bass_utils import run_bass_kernel_spmd

    value = np.asarray(inputs["value"], np.int32).astype(np.float32)
    depth = np.asarray(inputs["depth"], np.int32).astype(np.float32)
    position = np.asarray(inputs["position"], np.int32).astype(np.float32)

    tbl, sel, layers = _build_consts(inputs)
    nc = _get_nc(layers, tbl.shape[1] // E)

    in_maps = []
    for b in range(BATCH):
        rid = _build_ridx(value, depth, position, b)
        m = {"tbl": tbl, "sel": sel}
        for name, _, _, _ in layers:
            m[f"ridx_{name}"] = rid[name]
        in_maps.append(m)

    res = run_bass_kernel_spmd(nc, in_maps, list(range(BATCH)))
    return np.stack([res.results[b]["out"] for b in range(BATCH)])



# revision 10
# speedup vs baseline: 2.0879x; 2.0879x over previous
"""Trainium2 Bass kernel for nn_CompositeEmbeddingA (octree composite embedding).

Batch=8, one sample per NeuronCore (pure data parallel, no collectives).

Per sample:
  layers 0-2 (depths 1-3): x = val_emb[v] + pos0[p0] + pos1[p1] + pos2[p2] + dep_emb[d]
  layers 3-4: same sum w/o dep, then Conv1d(E,E,kernel=stride=k), k=4 (l3) / 8 (l4)

Formulation: every layer is out = OneHot @ Table on the PE.
  - depth is static per layer -> dep_emb row folded into val_emb rows.
  - conv folded into tables per tap j (T_j = table @ w[:,:,j].T); conv bias
    folded into tap-0 val rows (exactly one val row fires per tap).
  - index rows 0 of all tables are never referenced (indices are >= 1) and
    are dropped: 192 rows per (sub-layer | tap), so
    merged layers 0-2 ("B"): 576 rows -> 5 chunks of 128
    layer 3: 4 taps  -> 768 rows  -> 6 chunks
    layer 4: 8 taps  -> 1536 rows -> 12 chunks
  - the one-hot matrix (pure index preprocessing, no table data) is built
    host-side and shipped as fp8 (exact 0/1) in the DoubleRow k-tile layout.
  - tables are shipped as residual-compensated fp8 pairs (A = fp8(T/S),
    B = fp8(T/S - A)); each chunk is one fp8 DoubleRow matmul contracting
    (A, B) against the same one-hot (stride-0 k-tile broadcast), which costs
    half of a bf16 matmul per chunk at bf16-level accuracy (~1e-3 rel).
  - PSUM is evicted to bf16 with scale S, rotating DVE/ACT/Pool; output DMAs
    in bf16, host casts to f32 and reassembles token tiles.
"""

import sys

for _p in ("/opt/trn_rl_repo",):
    if _p not in sys.path:
        sys.path.insert(0, _p)

import numpy as np
import ml_dtypes

RES = 32
SPATIAL = 3
NUM_VOCAB = 3
E = 256
BATCH = 8
LAYER_SIZES = (8, 64, 512, 4096, 32768)
CONV_SIZE = {3: 4, 4: 8}
S_TOTAL = sum(LAYER_SIZES)  # 37448
OUT_TOKENS = 8 + 64 + 512 + 1024 + 4096  # 5704

_E8 = ml_dtypes.float8_e4m3fn
_BF16 = ml_dtypes.bfloat16

SCALE = 2.0**-9  # global table scale; folded back in at PSUM evict

ROWS_PER_GROUP = 3 + 63 * SPATIAL  # 192: val(3) + pos0/1/2(63 each)

# virtual layers: (name, token count T, n 128-row chunks, list of t-tile sizes)
def _tiles(T):
    return [min(128, T - t0) for t0 in range(0, T, 128)]

VLAYERS = (
    ("B", 584, 5),     # sub-layers 0-2 merged; 576 rows
    ("L3", 1024, 6),   # 768 rows
    ("L4", 4096, 12),  # 1536 rows
)
NCH_TOTAL = sum(nch for _, _, nch in VLAYERS)  # 23
NTT_TOTAL = sum(len(_tiles(T)) for _, T, _ in VLAYERS)  # 45
L4_STRIPE = 512  # tokens per L4 MH load


def _layer_slices():
    out = []
    start = 0
    for n in LAYER_SIZES:
        out.append((start, start + n))
        start += n
    return out


LAYER_SL = _layer_slices()


def _build_tables(params):
    """Residual-compensated fp8 table chunks.

    Returns tbl [128, NCH_TOTAL, 2, E] (fp8): per chunk, k-tile 0 = coarse
    fp8(T/S), k-tile 1 = fp8 residual.
    """
    rows = []

    def add_group(val3, pe):  # val3 [3,E], pe [SPATIAL, 64, E]
        rows.append(val3)
        for s in range(SPATIAL):
            rows.append(pe[s][1:64])

    # B: sub-layers 0..2, dep folded into val
    for l in range(3):
        val3 = (
            np.asarray(params[f"val_emb_{l}"], np.float32)[1:4]
            + np.asarray(params[f"dep_emb_{l}"], np.float32)[l + 1][None, :]
        )
        add_group(val3, np.asarray(params[f"pos_emb_{l}"], np.float32))
    # conv layers: per tap, tables folded through w[:,:,j]; bias into tap-0 val
    for l in (3, 4):
        w = np.asarray(params[f"conv_w_{l}"], np.float32)  # [O, E, k]
        b = np.asarray(params[f"conv_b_{l}"], np.float32)
        ve = np.asarray(params[f"val_emb_{l}"], np.float32)
        pe = np.asarray(params[f"pos_emb_{l}"], np.float32)
        for j in range(CONV_SIZE[l]):
            wj = w[:, :, j]
            val3 = ve[1:4] @ wj.T
            if j == 0:
                val3 = val3 + b[None, :]
            add_group(val3, pe @ wj.T)

    allrows = np.concatenate(rows, axis=0)  # [2880, E]
    assert allrows.shape[0] == 576 + 768 + 1536

    tbl = np.zeros((128, NCH_TOTAL, 2, E), np.float32)
    r0 = 0
    c0 = 0
    for _, _, nch in VLAYERS:
        n = {5: 576, 6: 768, 12: 1536}[nch]
        lay = np.zeros((nch * 128, E), np.float32)
        lay[:n] = allrows[r0 : r0 + n]
        r0 += n
        t = lay.reshape(nch, 128, E).transpose(1, 0, 2)  # [128, nch, E]
        a = (t / SCALE).astype(_E8).astype(np.float32)
        resid = (t / SCALE - a).astype(_E8).astype(np.float32)
        tbl[:, c0 : c0 + nch, 0, :] = a
        tbl[:, c0 : c0 + nch, 1, :] = resid
        c0 += nch
    return tbl.astype(_E8)


def _build_mh(value, position, b):
    """Host-built one-hot planes, fp8, chunk-major columns per virtual layer.

    Returns dict name -> [128, nch * T] fp8 where column c*T + t is chunk c,
    token t; row r fires iff global row id c*128+r is selected by token t.
    """
    out = {}

    def onehot(pairs, T, nch):
        # pairs: list of (gid_array, col_array); sets m[gid, col] = 1
        m = np.zeros((nch * 128, T), _E8)
        one = _E8(1.0)
        for g, c in pairs:
            m[g, c] = one
        return np.ascontiguousarray(
            m.reshape(nch, 128, T).transpose(1, 0, 2).reshape(128, nch * T)
        )

    # B: merged sub-layers; token cols 0..583 == input tokens 0..583
    pairs = []
    for l in range(3):
        lo, hi = LAYER_SL[l]
        cols = np.arange(lo, hi)
        base = ROWS_PER_GROUP * l
        pairs.append((base + (value[b, lo:hi] - 1), cols))
        for s in range(SPATIAL):
            pairs.append((base + 3 + 63 * s + (position[b, lo:hi, s] - 1), cols))
    out["B"] = onehot(pairs, 584, 5)

    for name, l in (("L3", 3), ("L4", 4)):
        k = CONV_SIZE[l]
        lo, hi = LAYER_SL[l]
        T = (hi - lo) // k
        nch = 6 if l == 3 else 12
        cols = np.arange(T)
        pairs = []
        for j in range(k):
            base = ROWS_PER_GROUP * j
            pairs.append((base + (value[b, lo:hi][j::k] - 1), cols))
            for s in range(SPATIAL):
                pairs.append((base + 3 + 63 * s + (position[b, lo:hi, s][j::k] - 1), cols))
        out[name] = onehot(pairs, T, nch)
    return out


_CACHE = {}


def _get_nc():
    key = "v2"
    if key in _CACHE:
        return _CACHE[key]

    import concourse.bass as bass
    import concourse.tile as tile
    from concourse import bacc, mybir
    from contextlib import ExitStack

    f32 = mybir.dt.float32
    bf16 = mybir.dt.bfloat16
    fp8 = mybir.dt.float8e4
    DR = mybir.MatmulPerfMode.DoubleRow
    A = mybir.ActivationFunctionType

    nc = bacc.Bacc(trn_type="TRN2", target_bir_lowering=False, debug=False)

    tbl_d = nc.dram_tensor("tbl", [128, NCH_TOTAL * 2 * E], fp8, kind="ExternalInput").ap()
    mh_d = {
        name: nc.dram_tensor(f"mh_{name}", [128, nch * T], fp8, kind="ExternalInput").ap()
        for name, T, nch in VLAYERS
    }
    out_d = nc.dram_tensor("out", [128, NTT_TOTAL * E], bf16, kind="ExternalOutput").ap()

    # chunk offset of each vlayer in tbl
    coff = {}
    c0 = 0
    for name, _, nch in VLAYERS:
        coff[name] = c0
        c0 += nch

    with tile.TileContext(nc) as tc, ExitStack() as ctx:
        cpool = ctx.enter_context(tc.tile_pool(name="const", bufs=1))
        psum = ctx.enter_context(tc.tile_pool(name="ps", bufs=3, space=bass.MemorySpace.PSUM))
        opool = ctx.enter_context(tc.tile_pool(name="osb", bufs=2))

        tbl_t = cpool.tile([128, NCH_TOTAL, 2, E], fp8, tag="tbl", name="tbl_t")
        tbl_v = tbl_d[:].rearrange("p (c k e) -> p c k e", k=2, e=E)

        def load_tbl(name):
            lo = coff[name]
            hi = lo + dict((n, c) for n, _, c in VLAYERS)[name]
            nc.sync.dma_start(tbl_t[:, lo:hi], tbl_v[:, lo:hi])

        # MH stripes: (vlayer name, token start, width, sbuf tile)
        mh_tiles = {}

        def load_mh(name, s0, W):
            T = dict((n, t) for n, t, _ in VLAYERS)[name]
            nch = dict((n, c) for n, _, c in VLAYERS)[name]
            t_ = cpool.tile([128, nch, W], fp8, tag=f"mh_{name}_{s0}", name="mh_t")
            src = mh_d[name][:].rearrange("p (c t) -> p c t", t=T)[:, :, s0 : s0 + W]
            nc.sync.dma_start(t_[:], src)
            mh_tiles[(name, s0)] = t_

        # ---- DMA emission order (SP queue, FIFO == transfer order) ----
        load_tbl("L4")
        load_mh("L4", 0, L4_STRIPE)
        load_tbl("B")
        load_mh("B", 0, 584)
        load_tbl("L3")
        load_mh("L3", 0, 1024)
        for s0 in range(L4_STRIPE, 4096, L4_STRIPE):
            load_mh("L4", s0, L4_STRIPE)

        # ---- compute ----
        # global t-tile index -> out column space
        evict_rr = [0]
        ENGS = ("vector", "scalar")  # gpsimd cannot access PSUM (BIR verifier)

        def evict(dst_ap, src_ap):
            eng = ENGS[evict_rr[0] % len(ENGS)]
            evict_rr[0] += 1
            if eng == "scalar":
                nc.scalar.activation(dst_ap, src_ap, A.Copy, scale=float(SCALE))
            elif eng == "vector":
                nc.vector.tensor_scalar(dst_ap, src_ap, float(SCALE), None, op0=mybir.AluOpType.mult)
            else:
                nc.gpsimd.tensor_scalar(dst_ap, src_ap, float(SCALE), None, op0=mybir.AluOpType.mult)

        gtile = [0]  # global t-tile counter (out column block index)
        out_dmas = []

        def do_layer(name, s0, W, ob, ob_g0):
            """mains+evicts for one loaded MH stripe into out staging tile ob
            (whose column block 0 corresponds to global tile ob_g0)."""
            nch = dict((n, c) for n, _, c in VLAYERS)[name]
            mh = mh_tiles[(name, s0)]
            tiles = _tiles(W)
            # process in bank pairs
            ti = 0
            while ti < len(tiles):
                pair = tiles[ti : ti + 2]
                pt = psum.tile([128, 512], f32, tag="o", name="pt")
                for h, M in enumerate(pair):
                    t0 = ti * 128 + h * 128
                    for c in range(nch):
                        lhs = mh[:, c : c + 1, t0 : t0 + M].broadcast_to((128, 2, M))
                        nc.tensor.matmul(
                            pt[:M, h * E : (h + 1) * E],
                            lhs,
                            tbl_t[:, coff[name] + c],
                            start=(c == 0),
                            stop=(c == nch - 1),
                            perf_mode=DR,
                        )
                g = gtile[0]
                col = (g - ob_g0) * E
                if len(pair) == 2 and pair[0] == 128 and pair[1] == 128:
                    evict(ob[:, col : col + 2 * E], pt[:])
                else:
                    for h, M in enumerate(pair):
                        evict(ob[:M, col + h * E : col + (h + 1) * E], pt[:M, h * E : (h + 1) * E])
                gtile[0] += len(pair)
                ti += len(pair)

        def flush_out(ob, g0, tiles):
            nfull = sum(1 for M in tiles if M == 128)
            if nfull:
                nc.scalar.dma_start(
                    out_d[:, g0 * E : (g0 + nfull) * E], ob[:, : nfull * E]
                )
            for i in range(nfull, len(tiles)):
                M = tiles[i]
                nc.scalar.dma_start(
                    out_d[:M, (g0 + i) * E : (g0 + i + 1) * E],
                    ob[:M, i * E : (i + 1) * E],
                )

        # L4 stripe 0 (4 tiles), then B (5), L3 (8), then L4 stripes 1..7
        # out staging: one tile per segment emitted
        def staged(name, s0, W):
            tiles = _tiles(W)
            g0 = gtile[0]
            ob = opool.tile([128, 8 * E], bf16, tag="ob", name="ob")
            do_layer(name, s0, W, ob, g0)
            flush_out(ob, g0, tiles)

        staged("L4", 0, L4_STRIPE)
        staged("B", 0, 584)
        staged("L3", 0, 1024)
        for s0 in range(L4_STRIPE, 4096, L4_STRIPE):
            staged("L4", s0, L4_STRIPE)

    nc.compile()
    _CACHE[key] = nc
    return nc


def kernel(**inputs):
    from concourse.bass_utils import run_bass_kernel_spmd

    value = np.asarray(inputs["value"], np.int64)
    position = np.asarray(inputs["position"], np.int64)

    tbl = _build_tables(inputs)
    tbl_flat = np.ascontiguousarray(tbl.reshape(128, NCH_TOTAL * 2 * E))
    nc = _get_nc()

    in_maps = []
    for b in range(BATCH):
        mh = _build_mh(value, position, b)
        m = {"tbl": tbl_flat}
        for name, T, nch in VLAYERS:
            m[f"mh_{name}"] = mh[name]
        in_maps.append(m)

    res = run_bass_kernel_spmd(nc, in_maps, list(range(BATCH)))

    outs = []
    for b in range(BATCH):
        o = np.asarray(res.results[b]["out"]).astype(np.float32)  # [128, 45*E]
        o = o.reshape(128, NTT_TOTAL, E).transpose(1, 0, 2)  # [45, 128, E]
        # global tile order: L4s0(4) B(5) L3(8) L4s1..7(28)
        l4_tiles = np.concatenate([o[0:4], o[17:45]], axis=0).reshape(-1, E)[:4096]
        b_tiles = o[4:9].reshape(-1, E)[:584]
        l3_tiles = o[9:17].reshape(-1, E)[:1024]
        outs.append(np.concatenate([b_tiles, l3_tiles, l4_tiles], axis=0))
    return np.stack(outs)
